# revision 33
# baseline (speedup 1.0000x reference)
"""BayesianDense (training path) Trainium2 kernel.

Computes, for B=512, D=512, O=256:
    sigma  = exp(W_log_sigma / 2)                     (D, O)
    out[b] = x[b] @ W_mu
           + sum_d x[b,d] * sigma[d,:] * e[b,d,:]     (noise matvec)
           + b_mu + eb[b] * exp(b_log_sigma / 2)

Data-parallel over batch across 8 NeuronCores (64 examples/core).
The dominant cost is streaming e (256 MB total, 32 MB/core) from HBM,
so the kernel is built to run at the HBM roofline (~358 GB/s/core):
  - Flat D-split: d = 4*a + j with a the SBUF partition, (j, o) free —
    every e/W DMA moves 4 KB contiguous runs per partition.
  - Each 4 MB chunk (8 examples) is split across the two HWDGE rings
    (SP + ACT sequencers); const loads ride the SWDGE (gpsimd) queue.
  - DVE does one full-width tensor_mul per example: t = e_b * sigma.
  - PE reduces over d per example; two variants:
      fp32t: exact fp32, transposed output — t blocks are the
             *stationary* operand, the x column streams (N=1), so
             fp32's 4 cyc/row stream penalty is negligible. Results
             land as outT[o, b] columns; a PE transpose at the end
             restores [b, o].
      fp32r: single-pass fp32 (TF32-like rounding), t streams as the
             moving operand (N=256), per-example rows land on PSUM
             partition 0 and are re-scattered by an SBUF->SBUF DMA.
"""

import numpy as np

B, D, O = 512, 512, 256
NCORES = 8
BL = B // NCORES          # 64 examples per core
P = 128                   # SBUF partitions
ND = D // P               # 4 d-blocks (j) of the flat split d = 4a + j
NH = O // P               # 2 o-halves for the transposed-output path
CHUNK = 8                 # examples per e-DMA chunk
NCHUNK = BL // CHUNK      # 8 chunks per core

# Reduction variants (measured on HW, 8 cores):
#   "fp32"  : exact fp32 matvecs (4 cyc/row stream)   ~121 us, rel ~3e-6
#   "fp32r" : TF32-like single-pass matvecs           ~106 us, rel ~1.2e-4
#   "fp32t" : exact fp32, stationary-t transposed     ~225 us (ldweights-bound)
MATMUL_MODE = "fp32"

_cache = {}


def _build(reps=1, mode=None):
    import concourse.mybir as mybir
    import concourse.tile as tile
    from concourse import bacc

    mode = mode or MATMUL_MODE
    f32 = mybir.dt.float32
    f32r = mybir.dt.float32r
    Exp = mybir.ActivationFunctionType.Exp
    Copy = mybir.ActivationFunctionType.Copy

    nc = bacc.Bacc("TRN2", target_bir_lowering=False, debug=False,
                   num_devices=NCORES)

    e_d = nc.dram_tensor("e", [BL, D, O], f32, kind="ExternalInput").ap()
    xT_d = nc.dram_tensor("xT", [D, BL], f32, kind="ExternalInput").ap()
    wmu_d = nc.dram_tensor("W_mu", [D, O], f32, kind="ExternalInput").ap()
    wls_d = nc.dram_tensor("W_ls", [D, O], f32, kind="ExternalInput").ap()
    if mode == "fp32t":
        ebT_d = nc.dram_tensor("ebT", [O, BL], f32, kind="ExternalInput").ap()
        bmu_d = nc.dram_tensor("bmu_col", [O, 1], f32, kind="ExternalInput").ap()
        bls_d = nc.dram_tensor("bls_col", [O, 1], f32, kind="ExternalInput").ap()
        id_d = nc.dram_tensor("id128", [P, P], f32, kind="ExternalInput").ap()
    else:
        eb_d = nc.dram_tensor("eb", [BL, O], f32, kind="ExternalInput").ap()
        bmu_d = nc.dram_tensor("bmu64", [BL, O], f32, kind="ExternalInput").ap()
        bls_d = nc.dram_tensor("bls64", [BL, O], f32, kind="ExternalInput").ap()
    out_d = nc.dram_tensor("out", [BL, O], f32, kind="ExternalOutput").ap()

    ps_bufs = 2 if mode == "fp32t" else 6
    with tile.TileContext(nc) as tc:
        with tc.tile_pool(name="const", bufs=1) as cpool, \
             tc.tile_pool(name="chunks", bufs=3) as chpool, \
             tc.tile_pool(name="prod", bufs=6) as tpool, \
             tc.tile_pool(name="psum", bufs=ps_bufs, space="PSUM") as pspool, \
             tc.tile_pool(name="psum_w", bufs=2, space="PSUM") as pwpool, \
             tc.tile_pool(name="psum_tr", bufs=2, space="PSUM") as ptpool:
          for _rep in range(reps):
            # ---- params (SWDGE queue; 4 KB-contiguous flat layout) -------
            sigma = cpool.tile([P, ND * O], f32)
            nc.gpsimd.dma_start(sigma[:].rearrange("a (j o) -> a j o", j=ND),
                                wls_d.rearrange("(a j) o -> a j o", a=P))
            nc.scalar.activation(sigma[:], sigma[:], Exp, scale=0.5)

            wmu = cpool.tile([P, ND * O], f32)
            nc.gpsimd.dma_start(wmu[:].rearrange("a (j o) -> a j o", j=ND),
                                wmu_d.rearrange("(a j) o -> a j o", a=P))

            xT = cpool.tile([P, ND * BL], f32)
            nc.gpsimd.dma_start(xT[:].rearrange("a (j b) -> a j b", j=ND),
                                xT_d.rearrange("(a j) b -> a j b", a=P))
            if mode == "fp32r":
                # fp32r matmul operands must be produced rounded-to-fp32r
                xTr = cpool.tile([P, ND * BL], f32r)
                nc.vector.tensor_copy(xTr[:], xT[:])
            elif mode == "fp32":
                xTr = xT

            e_r = e_d.rearrange("(c b) (a j) o -> c a b j o", b=CHUNK, a=P)

            if mode == "fp32t":
                # bias^T[o, b] = b_mu[o] + ebT[o, b] * exp(b_ls[o]/2):
                # one ACT op per o-half with per-partition scale+bias.
                id128 = cpool.tile([P, P], f32)
                nc.gpsimd.dma_start(id128[:], id_d[:, :])
                sigb = cpool.tile([P, NH], f32)
                nc.gpsimd.dma_start(
                    sigb[:], bls_d.rearrange("(h p) one -> p (h one)", p=P))
                nc.scalar.activation(sigb[:], sigb[:], Exp, scale=0.5)
                bmu = cpool.tile([P, NH], f32)
                nc.gpsimd.dma_start(
                    bmu[:], bmu_d.rearrange("(h p) one -> p (h one)", p=P))
                ebT = cpool.tile([P, NH * BL], f32)
                nc.gpsimd.dma_start(
                    ebT[:].rearrange("p (h b) -> p h b", h=NH),
                    ebT_d.rearrange("(h p) b -> p h b", p=P))
                biasT = cpool.tile([P, NH * BL], f32)
                for h in range(NH):
                    nc.vector.tensor_scalar(
                        out=biasT[:, h * BL:(h + 1) * BL],
                        in0=ebT[:, h * BL:(h + 1) * BL],
                        scalar1=sigb[:, h:h + 1],
                        scalar2=bmu[:, h:h + 1],
                        op0=mybir.AluOpType.mult,
                        op1=mybir.AluOpType.add)

                # x @ W_mu, transposed: outT_wmu[o-half] (128, 64)
                outT = cpool.tile([P, NH * BL], f32)
                ps_w = []
                for h in range(NH):
                    pw = pwpool.tile([P, BL], f32)
                    for j in range(ND):
                        nc.tensor.matmul(
                            pw[:, :],
                            lhsT=wmu[:, j * O + h * P: j * O + (h + 1) * P],
                            rhs=xT[:, j * BL:(j + 1) * BL],
                            start=(j == 0), stop=(j == ND - 1),
                        )
                    ps_w.append(pw)

                for c in range(NCHUNK):
                    ch = chpool.tile([P, CHUNK * ND * O], f32)
                    chv = ch[:].rearrange("a (b j o) -> a b j o",
                                          b=CHUNK, j=ND)
                    half = CHUNK // 2
                    nc.sync.dma_start(chv[:, :half], e_r[c][:, :half])
                    nc.scalar.dma_start(chv[:, half:], e_r[c][:, half:])

                    pst = [pspool.tile([P, CHUNK], f32,
                                       name=f"pst{h}", tag=f"pst{h}")
                           for h in range(NH)]
                    for b in range(CHUNK):
                        t = tpool.tile([P, ND * O], f32)
                        nc.vector.tensor_mul(
                            t[:], ch[:, b * ND * O:(b + 1) * ND * O], sigma[:])
                        bg = c * CHUNK + b
                        for j in range(ND):
                            xcol = xT[:, j * BL + bg: j * BL + bg + 1]
                            for h in range(NH):
                                nc.tensor.matmul(
                                    pst[h][:, b:b + 1],
                                    lhsT=t[:, j * O + h * P: j * O + (h + 1) * P],
                                    rhs=xcol,
                                    start=(j == 0), stop=(j == ND - 1),
                                    skip_group_check=True,
                                )
                    for h in range(NH):
                        nc.scalar.copy(
                            outT[:, h * BL + c * CHUNK:
                                 h * BL + (c + 1) * CHUNK], pst[h][:, :])

                # outT += wmu^T + bias^T, then transpose back to [b, o]
                out_sb = cpool.tile([BL, O], f32)
                for h in range(NH):
                    sl = outT[:, h * BL:(h + 1) * BL]
                    nc.vector.tensor_add(sl, sl, ps_w[h][:, :])
                    nc.vector.tensor_add(sl, sl, biasT[:, h * BL:(h + 1) * BL])
                    ptr = ptpool.tile([BL, P], f32)
                    nc.tensor.transpose(ptr[:, :], sl, id128[:])
                    nc.scalar.copy(out_sb[:, h * P:(h + 1) * P], ptr[:, :])
                nc.sync.dma_start(out_d[:, :], out_sb[:])

            else:  # fp32r
                sigb = cpool.tile([BL, O], f32)
                nc.gpsimd.dma_start(sigb[:], bls_d[:, :])
                nc.scalar.activation(sigb[:], sigb[:], Exp, scale=0.5)
                ebt = cpool.tile([BL, O], f32)
                nc.gpsimd.dma_start(ebt[:], eb_d[:, :])
                bmu = cpool.tile([BL, O], f32)
                nc.gpsimd.dma_start(bmu[:], bmu_d[:, :])
                bias = cpool.tile([BL, O], f32)
                nc.vector.tensor_mul(bias[:], ebt[:], sigb[:])
                nc.vector.tensor_add(bias[:], bias[:], bmu[:])

                ps_wmu = pwpool.tile([BL, O], f32)
                for j in range(ND):
                    nc.tensor.matmul(
                        ps_wmu[:, :],
                        lhsT=xT[:, j * BL:(j + 1) * BL],
                        rhs=wmu[:, j * O:(j + 1) * O],
                        start=(j == 0), stop=(j == ND - 1),
                    )
                # out_sb pre-filled with x@W_mu + bias; per-chunk noise rows
                # are scatter-accumulated on top, then stored — no serial tail.
                out_sb = cpool.tile([BL, O], f32)
                nc.scalar.copy(out_sb[:], ps_wmu[:, :])
                nc.vector.tensor_add(out_sb[:], out_sb[:], bias[:])

                stage = cpool.tile([1, BL * O], f32)
                stage_r = stage[:].rearrange("one (b o) -> one b o", b=BL)

                for c in range(NCHUNK):
                    ch = chpool.tile([P, CHUNK * ND * O], f32)
                    chv = ch[:].rearrange("a (b j o) -> a b j o",
                                          b=CHUNK, j=ND)
                    half = CHUNK // 2
                    if c == 0:
                        # fine-grained first fill: compute starts after one
                        # example (0.5 MB) instead of a whole 2 MB half
                        for b in range(CHUNK):
                            eng = nc.sync if b % 2 == 0 else nc.scalar
                            eng.dma_start(chv[:, b:b + 1], e_r[c][:, b:b + 1])
                    else:
                        nc.sync.dma_start(chv[:, :half], e_r[c][:, :half])
                        nc.scalar.dma_start(chv[:, half:], e_r[c][:, half:])
                    for b in range(CHUNK):
                        t = tpool.tile([P, ND * O],
                                       f32r if mode == "fp32r" else f32)
                        # Pool takes 2 of 8 multiplies to offload the DVE
                        mul_eng = nc.gpsimd if (mode == "fp32" and b in (3, 6)) \
                            else nc.vector
                        mul_eng.tensor_mul(
                            t[:], ch[:, b * ND * O:(b + 1) * ND * O], sigma[:])
                        bg = c * CHUNK + b
                        ps = pspool.tile([1, O], f32)
                        for j in range(ND):
                            nc.tensor.matmul(
                                ps[:, :],
                                lhsT=xTr[:, j * BL + bg: j * BL + bg + 1],
                                rhs=t[:, j * O:(j + 1) * O],
                                start=(j == 0), stop=(j == ND - 1),
                            )
                        nc.scalar.copy(stage[:, bg * O:(bg + 1) * O], ps[:, :])
                    # scatter-accumulate this chunk's rows and store them
                    nc.gpsimd.dma_start(
                        out_sb[c * CHUNK:(c + 1) * CHUNK, :],
                        stage_r[:, c * CHUNK:(c + 1) * CHUNK],
                        accum_op=mybir.AluOpType.add)
                    nc.sync.dma_start(out_d[c * CHUNK:(c + 1) * CHUNK, :],
                                      out_sb[c * CHUNK:(c + 1) * CHUNK, :])

    nc.compile()
    return nc


def _get_nc(reps=1, mode=None):
    key = ("nc", reps, mode or MATMUL_MODE)
    if key not in _cache:
        _cache[key] = _build(reps, mode)
    return _cache[key]


def _in_maps(x, W_mu, W_log_sigma, b_mu, b_log_sigma, e, eb, mode=None):
    mode = mode or MATMUL_MODE
    x = np.asarray(x, dtype=np.float32)
    W_mu = np.ascontiguousarray(W_mu, dtype=np.float32)
    W_ls = np.ascontiguousarray(W_log_sigma, dtype=np.float32)
    e = np.asarray(e, dtype=np.float32)
    eb = np.asarray(eb, dtype=np.float32)
    b_mu = np.asarray(b_mu, dtype=np.float32)
    b_ls = np.asarray(b_log_sigma, dtype=np.float32)
    maps = []
    for c in range(NCORES):
        sl = slice(c * BL, (c + 1) * BL)
        m = {
            "e": np.ascontiguousarray(e[sl]),
            "xT": np.ascontiguousarray(x[sl].T),
            "W_mu": W_mu,
            "W_ls": W_ls,
        }
        if mode == "fp32t":
            m["ebT"] = np.ascontiguousarray(eb[sl].T)
            m["bmu_col"] = np.ascontiguousarray(b_mu.reshape(O, 1))
            m["bls_col"] = np.ascontiguousarray(b_ls.reshape(O, 1))
            m["id128"] = np.eye(P, dtype=np.float32)
        else:
            m["eb"] = np.ascontiguousarray(eb[sl])
            m["bmu64"] = np.ascontiguousarray(
                np.broadcast_to(b_mu, (BL, O)), dtype=np.float32)
            m["bls64"] = np.ascontiguousarray(
                np.broadcast_to(b_ls, (BL, O)), dtype=np.float32)
        maps.append(m)
    return maps


def run(trace=False, reps=1, mode=None, **inputs):
    """Run on the 8 NeuronCores; returns (full_output, BassKernelResults)."""
    from concourse.bass_utils import run_bass_kernel_spmd

    nc = _get_nc(reps, mode)
    maps = _in_maps(**inputs, mode=mode)
    res = run_bass_kernel_spmd(nc, maps, list(range(NCORES)), trace=trace)
    out = np.concatenate([r["out"] for r in res.results], axis=0)
    return out, res


def kernel(**inputs) -> np.ndarray:
    out, _ = run(trace=False, **inputs)
    return out
